# revision 1
# baseline (speedup 1.0000x reference)
"""Trainium2 Bass kernel for nn_AttentionBase (8-core SPMD).

Math (see reference):
  headers = data[:, :100]; col_feat = data[:, 100:]
  sim[q,c] = (headers*w_cq) @ title.T + (headers@w_c+b_c)[q] + (title@w_q+b_q)[c] + b_cq
  t2q = Q * softmax(max_c sim) @ col_feat          # [400]
  q2t = C * softmax(max_q sim) @ title             # [100]
  x = [t2q q2t] -> 7-layer MLP -> [1, 8]

Distribution: Q (4096) row-sharded 8 ways.  Per core, sim' = cq + r is one
K=101 matmul per c-chunk ([c-part, q-free] tiles; the r rank-1 term rides an
extra contraction row, b_q is folded into r).  The t = title@w_q column comes
from a second N=1 matmul on the same stationary weights; its add is fused
into the DVE scans (scalar_tensor_tensor).  col-max = free-axis reduce (DVE);
row-max = fused add+max accumulation (DVE) + PE-transpose partition fold.
One AllGather shares the maxes; the MLP's big layers are col/row sharded
with AllGathers of the small partial vectors between the relu boundaries.

Container quirks honoured here: walrus rejects >1 semaphore wait per
instruction unless the Bacc finalize() pipeline (event semaphores) runs;
TensorTensor is rejected on the Pool engine (no GPSIMD elementwise offload);
compute engines may only address partition bases 0/32/64/96.
"""

import os
import sys

import numpy as np

sys.path.insert(0, "/opt/trn_rl_repo")

from concourse import bacc
import concourse.mybir as mybir
import concourse.tile as tile
from concourse.bass import ds, ts
from concourse.masks import make_identity
from bass_rust import add_dep_helper

F32 = mybir.dt.float32
AX = mybir.AxisListType
ALU = mybir.AluOpType
ACTF = mybir.ActivationFunctionType

C, D, Q, F = 8192, 100, 4096, 400
NC = 8
QS = Q // NC          # 512  q per core
NCHUNK = C // 128     # 64   c-chunks
NGROUP = NCHUNK // 2  # 32   groups of 2 chunks (one 2-bank psum mega-tile)
NEG = -1.0e30

def build_program():
    nc = bacc.Bacc(trn_type="TRN2", num_devices=NC)

    # ---------------- I/O ----------------
    title = nc.dram_tensor("title", [C, D], F32, kind="ExternalInput")
    dsh = nc.dram_tensor("data_shard", [QS, D + F], F32, kind="ExternalInput")
    auxv = nc.dram_tensor("auxv", [D, 4], F32, kind="ExternalInput")
    auxs = nc.dram_tensor("auxs", [1, 11], F32, kind="ExternalInput")
    bcol = nc.dram_tensor("bcol", [125, 34], F32, kind="ExternalInput")
    w1 = nc.dram_tensor("W1", [500, 500], F32, kind="ExternalInput")
    w2s = nc.dram_tensor("W2s", [500, 125], F32, kind="ExternalInput")
    w3s = nc.dram_tensor("W3s", [125, 3000], F32, kind="ExternalInput")
    w4s = nc.dram_tensor("W4s", [3000, 125], F32, kind="ExternalInput")
    w5s = nc.dram_tensor("W5s", [125, 500], F32, kind="ExternalInput")
    w6 = nc.dram_tensor("W6", [500, 100], F32, kind="ExternalInput")
    w7 = nc.dram_tensor("W7", [100, 8], F32, kind="ExternalInput")
    onehot = nc.dram_tensor("onehot", [NC, 1], F32, kind="ExternalInput")
    onesrow = nc.dram_tensor("onesrow", [1, C], F32, kind="ExternalInput")
    tmy = nc.dram_tensor("title_my", [C // NC, D], F32, kind="ExternalInput")
    out = nc.dram_tensor("out", [1, 8], F32, kind="ExternalOutput")

    with tile.TileContext(nc) as tc:
        with (
            tc.tile_pool(name="dram", bufs=1, space="DRAM") as dram,
            tc.tile_pool(name="consts", bufs=1) as consts,
            tc.tile_pool(name="big", bufs=1) as big,
            tc.tile_pool(name="scopy", bufs=4) as scopy,
            tc.tile_pool(name="small", bufs=1) as small,
        ):
            # ---- collective bounce buffers (DRAM) ----
            cc1_in = dram.tile([1, QS + C], F32, tag="cc1i")       # rowmax | colmax
            cc1_out = dram.tile([1, NC * (QS + C)], F32, tag="cc1o")
            cc2_in = dram.tile([125, 4], F32, tag="cc2i")
            cc2_out = dram.tile([NC, 500], F32, tag="cc2o")
            cc3_in = dram.tile([125, 24], F32, tag="cc3i")
            cc3_out = dram.tile([NC, 3000], F32, tag="cc3o")
            cc4_in = dram.tile([125, 4], F32, tag="cc4i")
            cc4_out = dram.tile([NC, 500], F32, tag="cc4o")

            # ---- constants / small inputs ----
            ident = consts.tile([128, 128], F32, tag="ident")
            make_identity(nc, ident[:])
            auxv_t = consts.tile([D, 4], F32, tag="auxv")
            nc.sync.dma_start(auxv_t[:], auxv[:, :])
            wcq_c, wc_c, wq_c, b6_t = (auxv_t[:, i:i + 1] for i in range(4))
            auxs_t = consts.tile([1, 11], F32, tag="auxs")
            nc.sync.dma_start(auxs_t[:], auxs[:, :])
            bc_t, bq_t, bcq_t = (auxs_t[:, i:i + 1] for i in range(3))
            b7_t = auxs_t[:, 3:11]
            bcol_t = consts.tile([125, 34], F32, tag="bcol")
            nc.sync.dma_start(bcol_t[:], bcol[:, :])
            b1_t = bcol_t[:, 0:4]
            b3_t = bcol_t[:, 4:28]
            b5_t = bcol_t[:, 28:32]
            b2_t = bcol_t[:, 32:33]
            b4_t = bcol_t[:, 33:34]
            oh_t = consts.tile([NC, 1], F32, tag="oh")
            nc.sync.dma_start(oh_t[:], onehot[:, :])
            ones8 = consts.tile([1, NC], F32, tag="ones8")
            nc.vector.memset(ones8[:], 1.0)
            ones128 = consts.tile([1, 128], F32, tag="ones128")
            nc.vector.memset(ones128[:], 1.0)

            # ---- big SBUF inputs (data first: it gates rhs_buf; title in
            # 8 slices so block-0 transposes start early; weights are loaded
            # by DMAs emitted after the phase-1 program) ----
            data_t = big.tile([128, 4, D + F], F32, tag="data")
            nc.sync.dma_start(
                data_t[:], dsh[:, :].rearrange("(k p) d -> p k d", p=128)
            )
            title_nat = big.tile([128, NCHUNK, D], F32, tag="title_nat")
            for q8 in range(8):
                nc.sync.dma_start(
                    title_nat[:, ts(q8, 8), :],
                    title[ds(1024 * q8, 1024), :]
                    .rearrange("(j p) d -> p j d", p=128))
            tmy_t = big.tile([128, NC, D], F32, tag="tmy")
            nc.sync.dma_start(
                tmy_t[:], tmy[:, :].rearrange("(j p) d -> p j d", p=128))
            w1_t = big.tile([100, 5, 500], F32, tag="w1")
            w2_t = big.tile([125, 4, 125], F32, tag="w2")
            w3_t = big.tile([125, 3000], F32, tag="w3")
            w4_t = big.tile([125, 24, 125], F32, tag="w4")
            w5_t = big.tile([125, 500], F32, tag="w5")
            w6_t = big.tile([125, 4, 100], F32, tag="w6")
            w7_t = consts.tile([100, 8], F32, tag="w7")

            # ---- phase-1 working buffers ----
            lhs_buf = big.tile([101, C], F32, tag="lhs")    # titleT | ones
            rhs_buf = big.tile([101, QS], F32, tag="rhs")   # hqT | r+bq
            acc_d = big.tile([128, 512], F32, tag="accd")  # DVE row-max acc
            colmax = big.tile([128, NCHUNK], F32, tag="colmax")
            nc.vector.memset(acc_d[:], NEG)
            # compute engines may only address partition bases 0/32/64/96, so
            # rows 100/101 of the K=102 operands are written via DMA from
            # base-0 staging rows.
            r_stage = big.tile([1, QS], F32, tag="r_stage")
            nc.scalar.dma_start(lhs_buf[100:101, :], onesrow[:, :])
            wq_ext = consts.tile([101, 1], F32, tag="wq_ext")
            nc.vector.memset(wq_ext[:], 0.0)
            nc.vector.tensor_copy(wq_ext[0:D, :], wq_c)

            bsum = consts.tile([1, 1], F32, tag="bsum")
            nc.vector.tensor_add(bsum[:], bc_t, bcq_t)
            nc.vector.tensor_add(bsum[:], bsum[:], bq_t)

            with (
                tc.tile_pool(name="psT", bufs=2, space="PSUM") as psT,
                tc.tile_pool(name="psMega", bufs=4, space="PSUM") as psM,
                tc.tile_pool(name="psC", bufs=2, space="PSUM") as psC,
            ):
                # pre-gate PE on ident (Pool) and auxv (DMA) so later
                # matmuls never need more than one new sync-wait each.
                pgate = psT.tile([1, 512], F32, tag="pt")
                nc.tensor.transpose(pgate[0:1, 0:NC], ident[0:NC, 0:1],
                                    ident[0:NC, 0:NC])
                nc.tensor.matmul(pgate[0:1, 0:1], auxv_t[:, 0:1],
                                 auxv_t[:, 0:1], start=True, stop=True)
                # -- headers transpose -> rhs rows 0-99; then r row; then *w_cq
                for k in range(4):
                    p = psT.tile([128, 512], F32, tag="pt")
                    nc.tensor.transpose(p[0:D, 0:128], data_t[:, k, 0:D], ident[:])
                    nc.scalar.copy(rhs_buf[0:D, ts(k, 128)], p[0:D, 0:128])
                pr = psT.tile([1, 512], F32, tag="pt")
                nc.tensor.matmul(pr[:, :], wc_c, rhs_buf[0:D, :],
                                 start=True, stop=True)
                nc.scalar.activation(r_stage[:], pr[:, :], ACTF.Identity,
                                     bias=bsum[:], scale=1.0)
                nc.scalar.dma_start(rhs_buf[100:101, :], r_stage[:])
                nc.vector.tensor_scalar(rhs_buf[0:D, :], rhs_buf[0:D, :],
                                        wcq_c, None, op0=ALU.mult)

                # -- per 512-col block: transpose 4 title chunks, then the
                # block's four S' tiles (S' = cq + r', no t term).  The t
                # column comes from a second matmul on the same stationary
                # weights and its add is fused into the DVE scans, so no
                # engine ever waits on a distant title DMA slice.
                block_tail = {}
                for b in range(16):
                    p = psT.tile([128, 512], F32, tag="pt")
                    for jj in range(4):
                        j = 4 * b + jj
                        tr = nc.tensor.transpose(p[0:D, ts(jj, 128)],
                                                 title_nat[:, j, :], ident[:])
                        # cap transpose run-ahead: block b's transposes wait
                        # (order-only) for block b-2's mains, so the PE FIFO
                        # never head-blocks on a distant title DMA slice.
                        if b >= 2:
                            add_dep_helper(tr.ins, block_tail[b - 2].ins,
                                           False, "transpose runahead cap")
                    nc.scalar.copy(lhs_buf[0:D, ts(b, 512)], p[0:D, :])
                    for j in range(4 * b, 4 * b + 4):
                        mega = psM.tile([128, 512], F32, tag="mega")
                        tcol = psC.tile([128, 1], F32, tag="tc")
                        mm = nc.tensor.matmul(mega[:], lhs_buf[:, ts(j, 128)],
                                              rhs_buf[:], start=True, stop=True)
                        nc.tensor.matmul(tcol[:], lhs_buf[:, ts(j, 128)],
                                         wq_ext[:], start=True, stop=True)
                        cm_tmp = scopy.tile([128, 1], F32, tag="cmt")
                        nc.vector.reduce_max(cm_tmp[:], mega[:], axis=AX.X)
                        nc.vector.tensor_scalar(colmax[:, j:j + 1], cm_tmp[:],
                                                tcol[:], None, op0=ALU.add)
                        nc.vector.scalar_tensor_tensor(
                            acc_d[:], mega[:], tcol[:], acc_d[:],
                            op0=ALU.add, op1=ALU.max)
                        if j % 4 == 3:
                            block_tail[b] = mm

                # -- fold accumulators -> row_max [1, 512]
                rmax = acc_d
                # MLP weight loads (consumed only after AllGather #1)
                nc.sync.dma_start(w1_t[:],
                                  w1[:, :].rearrange("(k p) m -> p k m", p=100))
                nc.sync.dma_start(w2_t[:],
                                  w2s[:, :].rearrange("(k p) m -> p k m", p=125))
                nc.sync.dma_start(w3_t[:], w3s[:, :])
                nc.sync.dma_start(w4_t[:],
                                  w4s[:, :].rearrange("(k p) m -> p k m", p=125))
                nc.sync.dma_start(w5_t[:], w5s[:, :])
                nc.sync.dma_start(w6_t[:],
                                  w6[:, :].rearrange("(k p) m -> p k m", p=125))
                nc.sync.dma_start(w7_t[:], w7[:, :])

                # partition-axis max via PE transpose + free-axis reduce
                # (compute engines cannot address partition bases not in
                # {0,32,64,96}, so no in-place halving tree).
                prt = psT.tile([128, 512], F32, tag="pt")
                for j in range(4):
                    nc.tensor.transpose(prt[:, ts(j, 128)],
                                        rmax[:, ts(j, 128)], ident[:])
                rmT = big.tile([128, 4], F32, tag="rmT")
                nc.vector.reduce_max(
                    rmT[:], prt[:, :].rearrange("p (a b) -> p a b", b=128),
                    axis=AX.X)
                # rmT[p, j] = row_max at local q = 128 j + p
                nc.scalar.dma_start(
                    cc1_in[0:1, 0:QS].rearrange("o (j p) -> (o p) j", p=128),
                    rmT[:])
                nc.scalar.dma_start(
                    cc1_in[0:1, QS:].rearrange("o (p j) -> (o p) j", p=128),
                    colmax[:])

            # ---- AllGather #1: maxes ----
            nc.gpsimd.collective_compute(
                "AllGather", ALU.bypass,
                replica_groups=[list(range(NC))],
                ins=[cc1_in[:, :].opt()], outs=[cc1_out[:, :].opt()])

            with tc.tile_pool(name="ps2", bufs=8, space="PSUM") as ps2:
                # ---- softmax over row-max (colw side) ----
                rm_all = small.tile([NC, QS], F32, tag="rm_all")
                nc.sync.dma_start(
                    rm_all[:],
                    cc1_out[0:1, :].rearrange("o (k x) -> (o k) x", k=NC)[:, 0:QS])
                m8 = small.tile([NC, 1], F32, tag="m8")
                nc.vector.reduce_max(m8[:], rm_all[:], axis=AX.X)
                pm8 = ps2.tile([1, 8], F32, tag="ps")
                nc.tensor.transpose(pm8[:], m8[:], ident[0:NC, 0:NC])
                negmr = small.tile([1, 1], F32, tag="negmr")
                nc.vector.reduce_max(negmr[:], pm8[:], axis=AX.X)
                nc.vector.tensor_scalar(negmr[:], negmr[:], -1.0, None,
                                        op0=ALU.mult)
                pb8 = ps2.tile([NC, 1], F32, tag="ps")
                nc.tensor.matmul(pb8[:], ones8[:], negmr[:], start=True, stop=True)
                negmr8 = small.tile([NC, 1], F32, tag="negmr8")
                nc.vector.tensor_copy(negmr8[:], pb8[:])
                e_all = small.tile([NC, QS], F32, tag="e_all")
                nc.scalar.activation(e_all[:], rm_all[:], ACTF.Exp,
                                     bias=negmr8[:], scale=1.0)
                d8 = small.tile([NC, 1], F32, tag="d8")
                nc.vector.reduce_sum(d8[:], e_all[:], axis=AX.X)
                pd8 = ps2.tile([1, 8], F32, tag="ps")
                nc.tensor.transpose(pd8[:], d8[:], ident[0:NC, 0:NC])
                sR = small.tile([1, 1], F32, tag="sR")
                nc.vector.reduce_sum(sR[:], pd8[:], axis=AX.X)
                nc.vector.reciprocal(sR[:], sR[:])
                nc.vector.tensor_scalar(sR[:], sR[:], float(Q), None, op0=ALU.mult)
                pb8b = ps2.tile([NC, 1], F32, tag="ps")
                nc.tensor.matmul(pb8b[:], ones8[:], sR[:], start=True, stop=True)
                ohs = small.tile([NC, 1], F32, tag="ohs")
                nc.vector.tensor_tensor(ohs[:], oh_t[:], pb8b[:], op=ALU.mult)

                # colw as columns [128, 4] (select own row of e_all, scaled)
                pcw = ps2.tile([128, 4], F32, tag="ps")
                for cb in range(4):
                    nc.tensor.matmul(pcw[:, cb:cb + 1], e_all[:, ts(cb, 128)],
                                     ohs[:], start=True, stop=True)
                colw = small.tile([128, 4], F32, tag="colw")
                nc.vector.tensor_copy(colw[:], pcw[:])

                # ---- softmax over col-max (titlew side) ----
                cm_raw = small.tile([128, NC, NCHUNK], F32, tag="cm_raw")
                nc.sync.dma_start(
                    cm_raw[:],
                    cc1_out[0:1, :].rearrange("o (k x) -> (o k) x", k=NC)
                    [:, QS:].rearrange("k (p j) -> p k j", p=128))
                cmax = small.tile([128, NCHUNK], F32, tag="cmax")
                nc.vector.tensor_tensor(
                    cm_raw[:, 0:4, :], cm_raw[:, 0:4, :], cm_raw[:, 4:8, :],
                    op=ALU.max)
                nc.vector.tensor_tensor(
                    cm_raw[:, 0:2, :], cm_raw[:, 0:2, :], cm_raw[:, 2:4, :],
                    op=ALU.max)
                nc.vector.tensor_tensor(
                    cmax[:], cm_raw[:, 0:1, :].rearrange("p a b -> p (a b)"),
                    cm_raw[:, 1:2, :].rearrange("p a b -> p (a b)"), op=ALU.max)
                m128 = small.tile([128, 1], F32, tag="m128")
                nc.vector.reduce_max(m128[:], cmax[:], axis=AX.X)
                pm128 = ps2.tile([1, 128], F32, tag="ps")
                nc.tensor.transpose(pm128[:], m128[:], ident[:])
                negmc = small.tile([1, 1], F32, tag="negmc")
                nc.vector.reduce_max(negmc[:], pm128[:], axis=AX.X)
                nc.vector.tensor_scalar(negmc[:], negmc[:], -1.0, None,
                                        op0=ALU.mult)
                pbc = ps2.tile([128, 1], F32, tag="ps")
                nc.tensor.matmul(pbc[:], ones128[:], negmc[:], start=True,
                                 stop=True)
                negmc128 = small.tile([128, 1], F32, tag="negmc128")
                nc.vector.tensor_copy(negmc128[:], pbc[:])
                ec = small.tile([128, NCHUNK], F32, tag="ec")
                nc.scalar.activation(ec[:], cmax[:], ACTF.Exp, bias=negmc128[:],
                                     scale=1.0)
                dc = small.tile([128, 1], F32, tag="dc")
                nc.vector.reduce_sum(dc[:], ec[:], axis=AX.X)
                pdc = ps2.tile([1, 128], F32, tag="ps")
                nc.tensor.transpose(pdc[:], dc[:], ident[:])
                sC = small.tile([1, 1], F32, tag="sC")
                nc.vector.reduce_sum(sC[:], pdc[:], axis=AX.X)
                nc.vector.reciprocal(sC[:], sC[:])
                nc.vector.tensor_scalar(sC[:], sC[:], float(C), None,
                                        op0=ALU.mult)
                pbc2 = ps2.tile([128, 1], F32, tag="ps")
                nc.tensor.matmul(pbc2[:], ones128[:], sC[:], start=True, stop=True)
                sC128 = small.tile([128, 1], F32, tag="sC128")
                nc.vector.tensor_copy(sC128[:], pbc2[:])
                titlew = small.tile([128, NCHUNK], F32, tag="titlew")
                nc.vector.tensor_scalar(titlew[:], ec[:], sC128[:], None,
                                        op0=ALU.mult)

                # ---- pooling ----
                pt2q = ps2.tile([100, 4], F32, tag="ps")
                for fs in range(4):
                    for k in range(4):
                        nc.tensor.matmul(
                            pt2q[:, fs:fs + 1],
                            data_t[:, k, ds(D + 100 * fs, 100)],
                            colw[:, k:k + 1],
                            start=(k == 0), stop=(k == 3))
                pid8 = nc.gpsimd.partition_id() * NC
                tw_own = small.tile([128, NC], F32, tag="tw_own")
                nc.gpsimd.dma_start(tw_own[:], titlew[:, ds(pid8, NC)])
                pq2t = ps2.tile([100, 1], F32, tag="ps")
                for j in range(NC):
                    nc.tensor.matmul(pq2t[:], tmy_t[:, j, :],
                                     tw_own[:, j:j + 1],
                                     start=(j == 0), stop=(j == NC - 1))
                x_col = small.tile([100, 5], F32, tag="x_col")
                nc.vector.tensor_copy(x_col[:, 0:4], pt2q[:])
                nc.vector.tensor_copy(x_col[:, 4:5], pq2t[:])

                # ---- W1 (replicated): y1 = x @ W1, partial over cores ----
                py1 = ps2.tile([125, 4], F32, tag="ps")
                for m in range(4):
                    for k in range(5):
                        nc.tensor.matmul(py1[:, m:m + 1],
                                         w1_t[:, k, ds(125 * m, 125)],
                                         x_col[:, k:k + 1],
                                         start=(k == 0), stop=(k == 4))
                y1s = small.tile([125, 4], F32, tag="y1s")
                nc.vector.tensor_copy(y1s[:], py1[:])
                nc.scalar.dma_start(cc2_in[:, :], y1s[:])

                nc.gpsimd.collective_compute(
                    "AllGather", ALU.bypass,
                    replica_groups=[list(range(NC))],
                    ins=[cc2_in[:, :].opt()], outs=[cc2_out[:, :].opt()])

                y1g = small.tile([125, NC, 4], F32, tag="y1g")
                nc.sync.dma_start(
                    y1g[:], cc2_out[:, :].rearrange("k (p m) -> p k m", p=125))
                nc.vector.tensor_tensor(y1g[:, 0:4, :], y1g[:, 0:4, :],
                                        y1g[:, 4:8, :], op=ALU.add)
                nc.vector.tensor_tensor(y1g[:, 0:2, :], y1g[:, 0:2, :],
                                        y1g[:, 2:4, :], op=ALU.add)
                x1 = small.tile([125, 4], F32, tag="x1")
                nc.vector.tensor_tensor(
                    x1[:], y1g[:, 0:1, :].rearrange("p a b -> p (a b)"),
                    y1g[:, 1:2, :].rearrange("p a b -> p (a b)"), op=ALU.add)
                nc.vector.tensor_tensor(x1[:], x1[:], b1_t, op=ALU.add)

                # ---- W2 shard -> x2s [125, 1] ----
                px2 = ps2.tile([125, 1], F32, tag="ps")
                for k in range(4):
                    nc.tensor.matmul(px2[:], w2_t[:, k, :], x1[:, k:k + 1],
                                     start=(k == 0), stop=(k == 3))
                x2s = small.tile([125, 1], F32, tag="x2s")
                nc.scalar.activation(x2s[:], px2[:], ACTF.Relu, bias=b2_t,
                                     scale=1.0)

                # ---- W3 shard -> y3 [125, 24] partial ----
                py3 = ps2.tile([125, 24], F32, tag="ps")
                for m in range(24):
                    nc.tensor.matmul(py3[:, m:m + 1], w3_t[:, ds(125 * m, 125)],
                                     x2s[:], start=True, stop=True)
                y3s = small.tile([125, 24], F32, tag="y3s")
                nc.vector.tensor_copy(y3s[:], py3[:])
                nc.scalar.dma_start(cc3_in[:, :], y3s[:])

                nc.gpsimd.collective_compute(
                    "AllGather", ALU.bypass,
                    replica_groups=[list(range(NC))],
                    ins=[cc3_in[:, :].opt()], outs=[cc3_out[:, :].opt()])

                y3g = small.tile([125, NC, 24], F32, tag="y3g")
                nc.sync.dma_start(
                    y3g[:], cc3_out[:, :].rearrange("k (p m) -> p k m", p=125))
                nc.vector.tensor_tensor(y3g[:, 0:4, :], y3g[:, 0:4, :],
                                        y3g[:, 4:8, :], op=ALU.add)
                nc.vector.tensor_tensor(y3g[:, 0:2, :], y3g[:, 0:2, :],
                                        y3g[:, 2:4, :], op=ALU.add)
                y3sum = small.tile([125, 24], F32, tag="y3sum")
                nc.vector.tensor_tensor(
                    y3sum[:], y3g[:, 0:1, :].rearrange("p a b -> p (a b)"),
                    y3g[:, 1:2, :].rearrange("p a b -> p (a b)"), op=ALU.add)
                x3 = small.tile([125, 24], F32, tag="x3")
                nc.vector.tensor_tensor(x3[:], y3sum[:], b3_t, op=ALU.add)
                nc.vector.tensor_scalar(x3[:], x3[:], 0.0, None, op0=ALU.max)

                # ---- W4 shard -> x4s [125, 1] ----
                px4 = ps2.tile([125, 1], F32, tag="ps")
                for k in range(24):
                    nc.tensor.matmul(px4[:], w4_t[:, k, :], x3[:, k:k + 1],
                                     start=(k == 0), stop=(k == 23))
                x4s = small.tile([125, 1], F32, tag="x4s")
                nc.scalar.activation(x4s[:], px4[:], ACTF.Relu, bias=b4_t,
                                     scale=1.0)

                # ---- W5 shard -> y5 [125, 4] partial ----
                py5 = ps2.tile([125, 4], F32, tag="ps")
                for m in range(4):
                    nc.tensor.matmul(py5[:, m:m + 1], w5_t[:, ds(125 * m, 125)],
                                     x4s[:], start=True, stop=True)
                y5s = small.tile([125, 4], F32, tag="y5s")
                nc.vector.tensor_copy(y5s[:], py5[:])
                nc.scalar.dma_start(cc4_in[:, :], y5s[:])

                nc.gpsimd.collective_compute(
                    "AllGather", ALU.bypass,
                    replica_groups=[list(range(NC))],
                    ins=[cc4_in[:, :].opt()], outs=[cc4_out[:, :].opt()])

                y5g = small.tile([125, NC, 4], F32, tag="y5g")
                nc.sync.dma_start(
                    y5g[:], cc4_out[:, :].rearrange("k (p m) -> p k m", p=125))
                nc.vector.tensor_tensor(y5g[:, 0:4, :], y5g[:, 0:4, :],
                                        y5g[:, 4:8, :], op=ALU.add)
                nc.vector.tensor_tensor(y5g[:, 0:2, :], y5g[:, 0:2, :],
                                        y5g[:, 2:4, :], op=ALU.add)
                x5 = small.tile([125, 4], F32, tag="x5")
                nc.vector.tensor_tensor(
                    x5[:], y5g[:, 0:1, :].rearrange("p a b -> p (a b)"),
                    y5g[:, 1:2, :].rearrange("p a b -> p (a b)"), op=ALU.add)
                nc.vector.tensor_tensor(x5[:], x5[:], b5_t, op=ALU.add)
                nc.vector.tensor_scalar(x5[:], x5[:], 0.0, None, op0=ALU.max)

                # ---- W6 -> x6 [100, 1]; W7 -> out [1, 8] ----
                px6 = ps2.tile([100, 1], F32, tag="ps")
                for k in range(4):
                    nc.tensor.matmul(px6[:], w6_t[:, k, :], x5[:, k:k + 1],
                                     start=(k == 0), stop=(k == 3))
                x6 = small.tile([100, 1], F32, tag="x6")
                nc.scalar.activation(x6[:], px6[:], ACTF.Relu, bias=b6_t,
                                     scale=1.0)
                pout = ps2.tile([1, 8], F32, tag="ps")
                nc.tensor.matmul(pout[:], x6[:], w7_t[:], start=True, stop=True)
                out_sb = small.tile([1, 8], F32, tag="out_sb")
                nc.vector.tensor_tensor(out_sb[:], pout[:], b7_t, op=ALU.add)
                nc.vector.tensor_scalar(out_sb[:], out_sb[:], 0.0, None,
                                        op0=ALU.max)
                nc.sync.dma_start(out[:, :], out_sb[:])

    nc.finalize()
    return nc


_NC_CACHE = None


def _get_program():
    global _NC_CACHE
    if _NC_CACHE is None:
        _NC_CACHE = build_program()
    return _NC_CACHE


def _in_maps(inputs):
    f = lambda a: np.ascontiguousarray(a, dtype=np.float32)
    title = f(inputs["title"])
    data = f(inputs["data"])
    auxv = np.stack(
        [f(inputs["w_cq"]), f(inputs["w_c"]), f(inputs["w_q"]),
         f(inputs["b6"])], axis=1)
    auxs = np.concatenate(
        [f(inputs["b_c"]).reshape(1), f(inputs["b_q"]).reshape(1),
         f(inputs["b_cq"]).reshape(1), f(inputs["b7"]).reshape(8)]
    ).reshape(1, 11)
    bcol_shared = np.concatenate(
        [f(inputs["b1"]).reshape(4, 125).T, f(inputs["b3"]).reshape(24, 125).T,
         f(inputs["b5"]).reshape(4, 125).T], axis=1)
    shared = {
        "title": title,
        "auxv": np.ascontiguousarray(auxv, dtype=np.float32),
        "auxs": np.ascontiguousarray(auxs, dtype=np.float32),
        "W1": f(inputs["W1"]),
        "W6": f(inputs["W6"]),
        "W7": f(inputs["W7"]),
        "onesrow": np.ones((1, C), dtype=np.float32),
    }
    W2, W3 = f(inputs["W2"]), f(inputs["W3"])
    W4, W5 = f(inputs["W4"]), f(inputs["W5"])
    b2, b4 = f(inputs["b2"]), f(inputs["b4"])
    maps = []
    for i in range(NC):
        m = dict(shared)
        m["data_shard"] = data[QS * i:QS * (i + 1)].copy()
        m["title_my"] = title[(C // NC) * i:(C // NC) * (i + 1)].copy()
        m["W2s"] = W2[:, 125 * i:125 * (i + 1)].copy()
        m["W3s"] = W3[125 * i:125 * (i + 1), :].copy()
        m["W4s"] = W4[:, 125 * i:125 * (i + 1)].copy()
        m["W5s"] = W5[125 * i:125 * (i + 1), :].copy()
        m["bcol"] = np.ascontiguousarray(np.concatenate(
            [bcol_shared,
             b2[125 * i:125 * (i + 1)].reshape(125, 1),
             b4[125 * i:125 * (i + 1)].reshape(125, 1)], axis=1),
            dtype=np.float32)
        oh = np.zeros((NC, 1), dtype=np.float32)
        oh[i, 0] = 1.0
        m["onehot"] = oh
        maps.append(m)
    return maps


def kernel(**inputs):
    from concourse import bass_utils
    nc = _get_program()
    res = bass_utils.run_bass_kernel_spmd(
        nc, _in_maps(inputs), core_ids=list(range(NC)),
        trace=bool(int(os.environ.get("KERNEL_TRACE", "0"))))
    kernel.last_results = res
    return np.asarray(res.results[0]["out"], dtype=np.float32)


if __name__ == "__main__":
    import reference
    inputs = {k: np.asarray(v) for k, v in reference.setup_inputs().items()}
    expected = np.asarray(reference.reference(**inputs))
    actual = kernel(**inputs)
    err = np.abs(actual - expected).max() / (np.abs(expected).max() + 1e-30)
    print("expected:", expected)
    print("actual  :", actual)
    print("Relative error:", err)

